# revision 2
# baseline (speedup 1.0000x reference)
"""Distributed causal attention kernel for TRN2 (8 NeuronCores).

Problem: nn_Attention (dense_transformer)
  multimodal_seq [2,2048,2048] + actions [2,512,2048] -> concat seq n=2560
  qkv projections (separate weights per stream), 16 heads x 128, causal
  attention, separate out-projections per stream.

Sharding: core c in 0..7 -> batch b = c//4, head-group g = c%4 (4 heads).
  - QKV projection: column-parallel (per-core weight slices), no comm.
  - Attention: fully local per (batch, head-group).
  - Out-projection: row-parallel partial sums + ReduceScatter over the
    4-core batch group; host concatenates the scattered row-slices.

All matmuls in bf16 (fp32 PSUM accumulation). Layout trick: host supplies
X^T so both projection operands are in natural SBUF layout; attention
computes S^T = K^T.T @ Q^T so the A@V contraction needs no transposes.
Softmax denominator: DVE accumulates exp(S^T) tiles into Esum, then one
small fp32r ones-matmul per (chunk, head) reduces over partitions.
"""

from contextlib import ExitStack

import numpy as np
import ml_dtypes

import concourse.bass as bass
import concourse.mybir as mybir
import concourse.tile as tile
from concourse import bacc

BF16 = mybir.dt.bfloat16
F32 = mybir.dt.float32
F32R = mybir.dt.float32r

B = 2
H = 16
D = 128
DIM = 2048
SEQ = 2048
ACT = 512
N = SEQ + ACT            # 2560
HPC = 4                  # heads per core
NC_CORES = 8
KT = DIM // 128          # 16 contraction k-tiles
IT = N // 128            # 20 row tiles
ICN = N // 512           # 5 row chunks of 512
RG = [[0, 1, 2, 3], [4, 5, 6, 7]]
SCALE = D ** -0.5
# ReduceScatter slabs (row_start, rows): last chunk split so the final,
# unoverlappable RS is small
SLABS = [(0, 512), (512, 512), (1024, 512), (1536, 512), (2048, 384), (2432, 128)]
SLAB_OF_IT = {}
for _s, (_r0, _rows) in enumerate(SLABS):
    for _it in range(_r0 // 128, (_r0 + _rows) // 128):
        SLAB_OF_IT[_it] = _s
SLAB_OUT_OFF = [sum(r // 4 for _, r in SLABS[:s]) for s in range(len(SLABS))]


def build():
    nc = bacc.Bacc("TRN2", target_bir_lowering=False, debug=False,
                   num_devices=NC_CORES)

    xt = nc.dram_tensor("xt", [128, KT, N], BF16, kind="ExternalInput").ap()
    wqk = nc.dram_tensor("wqk", [128, 8, KT, 128], BF16, kind="ExternalInput").ap()
    wqk_a = nc.dram_tensor("wqk_a", [128, 8, KT, 128], BF16, kind="ExternalInput").ap()
    wv = nc.dram_tensor("wv", [128, KT, 512], BF16, kind="ExternalInput").ap()
    wv_a = nc.dram_tensor("wv_a", [128, KT, 512], BF16, kind="ExternalInput").ap()
    wo = nc.dram_tensor("wo", [128, HPC, DIM], BF16, kind="ExternalInput").ap()
    wo_a = nc.dram_tensor("wo_a", [128, HPC, DIM], BF16, kind="ExternalInput").ap()
    tri = nc.dram_tensor("tri", [128, 128], BF16, kind="ExternalInput").ap()
    onesel32 = nc.dram_tensor("onesel32", [128, 4 * HPC], F32R,
                              kind="ExternalInput").ap()
    sel = nc.dram_tensor("sel", [4, 512], BF16, kind="ExternalInput").ap()
    out_ext = nc.dram_tensor("out", [N // 4, DIM], BF16, kind="ExternalOutput").ap()

    Exp = mybir.ActivationFunctionType.Exp

    with tile.TileContext(nc) as tc:
        with ExitStack() as outer:
            qk_pool = outer.enter_context(tc.tile_pool(name="qkp", bufs=1))
            v_pool = outer.enter_context(tc.tile_pool(name="vp", bufs=1))
            c_pool = outer.enter_context(tc.tile_pool(name="cp", bufs=1))
            dram = outer.enter_context(tc.tile_pool(name="dram", bufs=1, space="DRAM"))

            qk_sb = qk_pool.tile([128, 8, N], BF16, name="qk_sb")
            v_sb = v_pool.tile([128, IT, 512], BF16, name="v_sb")
            tri_sb = c_pool.tile([128, 128], BF16, name="tri_sb")
            onesel32_sb = c_pool.tile([128, 4 * HPC], F32R, name="onesel32_sb")
            sel_sb = c_pool.tile([4, 512], BF16, name="sel_sb")

            nc.scalar.dma_start(out=tri_sb[:], in_=tri[:])
            nc.scalar.dma_start(out=oneselb_sb[:], in_=oneselb[:])
            nc.scalar.dma_start(out=sel_sb[:], in_=sel[:])

            # one DRAM bounce per slab: no false WAR between RS(s) reads and
            # slab s+1 writes
            prts = [dram.tile([rows, DIM], BF16, name=f"prt{s}")
                    for s, (_, rows) in enumerate(SLABS)]
            rs_outs = [dram.tile([rows // 4, DIM], BF16, name=f"rs_out{s}")
                       for s, (_, rows) in enumerate(SLABS)]

            # startup barrier: aligns the 8 cores so later ReduceScatter
            # barriers don't re-expose launch skew
            bar_in = dram.tile([1, 64], BF16, name="bar_in")
            bar_out = dram.tile([8, 64], BF16, name="bar_out")
            nc.sync.dma_start(out=bar_in[:], in_=tri[:1, :64])
            nc.gpsimd.collective_compute(
                "AllGather", mybir.AluOpType.bypass,
                replica_groups=[[0, 1, 2, 3, 4, 5, 6, 7]],
                ins=[bar_in[:]], outs=[bar_out[:]])

            # ---- Phases A+B: load X^T, QK^T projection ----
            with ExitStack() as ph1:
                xt_pool = ph1.enter_context(tc.tile_pool(name="xtp", bufs=1))
                xt_sb = xt_pool.tile([128, KT, N], BF16, name="xt_sb")

                with ExitStack() as phb:
                    wpool = phb.enter_context(tc.tile_pool(name="wqkp", bufs=2))
                    pb = phb.enter_context(
                        tc.tile_pool(name="pb", bufs=6, space="PSUM"))
                    # weights for mb0 go on the scalar queue ahead of the xt
                    # bulk so the first matmul can start almost immediately
                    wts, wats = {}, {}
                    for mb in range(2):
                        wts[mb] = wpool.tile([128, KT, 128], BF16, tag="w",
                                             name=f"wt{mb}")
                        wats[mb] = wpool.tile([128, KT, 128], BF16, tag="wa",
                                              name=f"wat{mb}")
                        nc.scalar.dma_start(out=wts[mb][:], in_=wqk[:, mb])
                        nc.scalar.dma_start(out=wats[mb][:], in_=wqk_a[:, mb])
                    for k in range(KT):
                        nc.sync.dma_start(out=xt_sb[:, k, :], in_=xt[:, k, :])
                    for mb in range(8):
                        if mb not in wts:
                            wts[mb] = wpool.tile([128, KT, 128], BF16, tag="w",
                                                 name=f"wt{mb}")
                            wats[mb] = wpool.tile([128, KT, 128], BF16, tag="wa",
                                                  name=f"wat{mb}")
                            nc.scalar.dma_start(out=wts[mb][:], in_=wqk[:, mb])
                            nc.scalar.dma_start(out=wats[mb][:], in_=wqk_a[:, mb])
                        wt, wat = wts[mb], wats[mb]
                        ps = [pb.tile([128, 512], F32, tag="b", name=f"ps{mb}_{r}")
                              for r in range(5)]
                        for k in range(KT):
                            for r in range(4):
                                nc.tensor.matmul(
                                    ps[r][:], wt[:, k, :], xt_sb[:, k, 512 * r:512 * r + 512],
                                    start=(k == 0), stop=(k == KT - 1))
                            nc.tensor.matmul(
                                ps[4][:], wat[:, k, :], xt_sb[:, k, SEQ:N],
                                start=(k == 0), stop=(k == KT - 1))
                        for r in range(5):
                            nc.any.tensor_copy(
                                out=qk_sb[:, mb, 512 * r:512 * r + 512], in_=ps[r][:])

                # ---- Phase C: V projection (natural layout) ----
                with ExitStack() as phc:
                    wvp = phc.enter_context(tc.tile_pool(name="wvp", bufs=1))
                    pc = phc.enter_context(
                        tc.tile_pool(name="pc", bufs=3, space="PSUM"))
                    wv_sb = wvp.tile([128, KT, 512], BF16, name="wv_sb")
                    wv_a_sb = wvp.tile([128, KT, 512], BF16, name="wv_a_sb")
                    for k in range(KT):
                        nc.sync.dma_start(out=wv_sb[:, k, :], in_=wv[:, k, :])
                    for k in range(KT):
                        nc.sync.dma_start(out=wv_a_sb[:, k, :], in_=wv_a[:, k, :])
                    for it in range(IT):
                        p = pc.tile([128, 512], F32, tag="c", name=f"vps{it}")
                        w = wv_sb if it < 16 else wv_a_sb
                        for k in range(KT):
                            nc.tensor.matmul(
                                p[:], xt_sb[:, k, 128 * it:128 * it + 128], w[:, k, :],
                                start=(k == 0), stop=(k == KT - 1))
                        nc.any.tensor_copy(out=v_sb[:, it, :], in_=p[:])

            # ---- Phases D+E: attention + out-projection + ReduceScatter ----
            with ExitStack() as ph2:
                wop = ph2.enter_context(tc.tile_pool(name="wop", bufs=1))
                o_pool = ph2.enter_context(tc.tile_pool(name="op", bufs=1))
                pd = ph2.enter_context(tc.tile_pool(name="pd", bufs=1, space="PSUM"))
                epool = ph2.enter_context(tc.tile_pool(name="ep", bufs=8))
                spool = ph2.enter_context(tc.tile_pool(name="sp", bufs=3))
                rpool = ph2.enter_context(tc.tile_pool(name="rp", bufs=2))

                wo_sb = wop.tile([128, HPC, DIM], BF16, name="wo_sb")
                wo_a_sb = wop.tile([128, HPC, DIM], BF16, name="wo_a_sb")
                nc.sync.dma_start(out=wo_sb[:], in_=wo[:])
                nc.sync.dma_start(out=wo_a_sb[:], in_=wo_a[:])
                o_sb = o_pool.tile([128, HPC, N], BF16, name="o_sb")

                for ic in range(ICN):
                    njt = 4 * ic + 4
                    o_ps = [pd.tile([128, 512], F32, tag=f"o{h}", name=f"ops{ic}_{h}")
                            for h in range(HPC)]
                    esum = [rpool.tile([128, 512], F32R, tag=f"es{h}",
                                       name=f"es{ic}_{h}") for h in range(HPC)]
                    for jt in range(njt):
                        diag = jt >= 4 * ic
                        st = (jt - 4 * ic) * 128 if diag else 0
                        for h in range(HPC):
                            s_ps = pd.tile([128, 512], F32, tag="s", bufs=4,
                                           name=f"sps{ic}_{jt}_{h}")
                            nc.tensor.matmul(
                                s_ps[:, st:],
                                qk_sb[:, 4 + h, 128 * jt:128 * jt + 128],
                                qk_sb[:, h, 512 * ic + st:512 * ic + 512],
                                start=True, stop=True)
                            eh = epool.tile([128, 512], BF16, tag="e",
                                            name=f"eh{ic}_{jt}_{h}")
                            if not diag:
                                nc.scalar.activation(eh[:], s_ps[:], Exp)
                            else:
                                nc.scalar.activation(
                                    eh[:, st:st + 128], s_ps[:, st:st + 128], Exp)
                                nc.vector.tensor_mul(
                                    out=eh[:, st:st + 128],
                                    in0=eh[:, st:st + 128], in1=tri_sb[:])
                                if st + 128 < 512:
                                    nc.scalar.activation(
                                        eh[:, st + 128:], s_ps[:, st + 128:], Exp)
                            nc.tensor.matmul(
                                o_ps[h][:, st:], v_sb[:, jt, 128 * h:128 * h + 128],
                                eh[:, st:],
                                start=(jt == 0), stop=(jt == njt - 1))
                            if jt == 0:
                                nc.vector.tensor_copy(out=esum[h][:], in_=eh[:])
                            else:
                                nc.vector.tensor_add(
                                    out=esum[h][:, st:], in0=esum[h][:, st:],
                                    in1=eh[:, st:])
                    # denominator: bf16 snapshot of esum (rounding after
                    # accumulation: ~0.03% on the colsum) -> fast bf16 matmul
                    den_ps = pd.tile([4, 512], F32, tag="s", bufs=4,
                                     name=f"den{ic}")
                    esb = []
                    for h in range(HPC):
                        e16 = spool.tile([128, 512], BF16, tag="esb", bufs=2,
                                         name=f"esb{ic}_{h}")
                        nc.vector.tensor_copy(out=e16[:], in_=esum[h][:])
                        esb.append(e16)
                    for h in range(HPC):
                        nc.tensor.matmul(
                            den_ps[:],
                            oneselb_sb[:, 4 * h:4 * h + 4],
                            esb[h][:],
                            start=(h == 0), stop=(h == HPC - 1))
                    rdenb = rpool.tile([4, 512], BF16, tag="rdenb", name=f"rdenb{ic}")
                    with nc.allow_low_precision("bf16 softmax denom broadcast"):
                        nc.vector.reciprocal(rdenb[:], den_ps[:])
                    for h in range(HPC):
                        bc_ps = pd.tile([128, 512], F32, tag="s", bufs=4,
                                        name=f"bc{ic}_{h}")
                        nc.tensor.matmul(
                            bc_ps[:], sel_sb[:, 128 * h:128 * h + 128], rdenb[:],
                            start=True, stop=True)
                        bc_sb = spool.tile([128, 512], F32, tag="bcs",
                                           name=f"bcs{ic}_{h}")
                        nc.any.tensor_copy(out=bc_sb[:], in_=bc_ps[:])
                        nc.vector.tensor_mul(
                            out=o_sb[:, h, 512 * ic:512 * ic + 512],
                            in0=o_ps[h][:], in1=bc_sb[:])
                    # out-projection for this row chunk
                    for t in range(4):
                        it = 4 * ic + t
                        s = SLAB_OF_IT[it]
                        r_off = 128 * it - SLABS[s][0]
                        wsrc = wo_sb if it < 16 else wo_a_sb
                        for ec in range(4):
                            pe_ps = pd.tile([128, 512], F32, tag=f"o{ec}",
                                            name=f"pe{it}_{ec}")
                            for h in range(HPC):
                                nc.tensor.matmul(
                                    pe_ps[:],
                                    o_sb[:, h, 128 * it:128 * it + 128],
                                    wsrc[:, h, 512 * ec:512 * ec + 512],
                                    start=(h == 0), stop=(h == HPC - 1))
                            stg = spool.tile([128, 512], BF16, tag="stg",
                                             name=f"stg{it}_{ec}")
                            nc.any.tensor_copy(out=stg[:], in_=pe_ps[:])
                            nc.sync.dma_start(
                                out=prts[s][r_off:r_off + 128,
                                            512 * ec:512 * ec + 512],
                                in_=stg[:])
                        if 128 * it + 128 == SLABS[s][0] + SLABS[s][1]:
                            # slab complete: reduce-scatter it and ship out
                            nc.gpsimd.collective_compute(
                                "ReduceScatter",
                                mybir.AluOpType.add,
                                replica_groups=RG,
                                ins=[prts[s][:]],
                                outs=[rs_outs[s][:]],
                            )
                            oo = SLAB_OUT_OFF[s]
                            nc.gpsimd.dma_start(
                                out=out_ext[oo:oo + SLABS[s][1] // 4, :],
                                in_=rs_outs[s][:])

    nc.compile()
    return nc


# ---------------- host-side sharding / unsharding ----------------

def _bf(a):
    return np.ascontiguousarray(a.astype(ml_dtypes.bfloat16))


def make_in_maps(multimodal_seq, actions, w_qkv, w_out, w_a_qkv, w_a_out):
    multimodal_seq = np.asarray(multimodal_seq, np.float32)
    actions = np.asarray(actions, np.float32)
    w_qkv = np.asarray(w_qkv, np.float32)
    w_out = np.asarray(w_out, np.float32)
    w_a_qkv = np.asarray(w_a_qkv, np.float32)
    w_a_out = np.asarray(w_a_out, np.float32)

    tri = np.triu(np.ones((128, 128), np.float32))  # tri[j,i] = 1 iff i >= j
    onesel32 = np.zeros((128, 4 * HPC), np.float32)
    for h in range(HPC):
        onesel32[:, 4 * h + h] = 1.0
    selm = np.zeros((4, 512), np.float32)
    for h in range(HPC):
        selm[h, 128 * h:128 * h + 128] = 1.0

    def w_slices(w, g):
        q = w[:, 512 * g:512 * g + 512] * SCALE
        k = w[:, SEQ + 512 * g:SEQ + 512 * g + 512]
        v = w[:, 2 * SEQ + 512 * g:2 * SEQ + 512 * g + 512]
        qk = np.concatenate([q, k], axis=1)                      # [2048, 1024]
        qk = qk.reshape(KT, 128, 8, 128).transpose(1, 2, 0, 3)   # [128,8,KT,128]
        vv = v.reshape(KT, 128, 512).transpose(1, 0, 2)          # [128,KT,512]
        return qk, vv

    in_maps = []
    for c in range(NC_CORES):
        b, g = divmod(c, 4)
        X = np.concatenate([multimodal_seq[b], actions[b]], axis=0)  # [2560, 2048]
        xt = X.T.reshape(KT, 128, N).transpose(1, 0, 2)              # [128,KT,2560]
        qk, vv = w_slices(w_qkv, g)
        qk_a, vv_a = w_slices(w_a_qkv, g)
        wo_h = w_out[512 * g:512 * g + 512].reshape(HPC, 128, DIM).transpose(1, 0, 2)
        wo_a_h = w_a_out[512 * g:512 * g + 512].reshape(HPC, 128, DIM).transpose(1, 0, 2)
        in_maps.append({
            "xt": _bf(xt),
            "wqk": _bf(qk), "wqk_a": _bf(qk_a),
            "wv": _bf(vv), "wv_a": _bf(vv_a),
            "wo": _bf(wo_h), "wo_a": _bf(wo_a_h),
            "tri": _bf(tri),
            "oneselb": _bf(onesel32),
            "sel": _bf(selm),
        })
    return in_maps


def assemble(results):
    """results: list of 8 dicts with 'out' [N//4, DIM] bf16 -> (mout, aout)."""
    mout = np.empty((B, SEQ, DIM), np.float32)
    aout = np.empty((B, ACT, DIM), np.float32)
    for b in range(B):
        full = np.empty((N, DIM), np.float32)
        for k in range(4):
            o = np.asarray(results[4 * b + k]["out"], np.float32)  # [N//4, DIM]
            for s, (r0, rows) in enumerate(SLABS):
                q = rows // 4
                full[r0 + k * q:r0 + (k + 1) * q] = \
                    o[SLAB_OUT_OFF[s]:SLAB_OUT_OFF[s] + q]
        mout[b] = full[:SEQ]
        aout[b] = full[SEQ:]
    return mout, aout


# ---------------- public entry point ----------------

_NC = None


def _get_compiled():
    global _NC
    if _NC is None:
        _NC = build()
    return _NC


def kernel(multimodal_seq, actions, w_qkv, w_out, w_a_qkv, w_a_out):
    """Full inputs in, full outputs out: returns (mout, aout) float32."""
    from concourse.bass_utils import run_bass_kernel_spmd

    nc = _get_compiled()
    in_maps = make_in_maps(multimodal_seq, actions, w_qkv, w_out,
                           w_a_qkv, w_a_out)
    res = run_bass_kernel_spmd(nc, in_maps, list(range(NC_CORES)))
    return assemble(res.results)
